# revision 2
# baseline (speedup 1.0000x reference)
"""DeepInterestNetwork (DIN) forward — Trainium2 Bass kernel, 8-core SPMD.

Distribution: pure data-parallel over the batch (4096 -> 512 per core).
The full embedding table (flattened [20*200001, 64]) is passed to every
core and gathered on-device via indirect DMA.

Per-core dataflow (all activations feature-major "transposed" layout so the
PE can contract over features):
  gather emb rows [10240, 64] (f-major) + hist rows [9728, 64] (l-major)
  PE-transpose pairs -> XT [1408, 512] slots / histT2 [128, 10*512] / quT2
  attention MLP (3 matmul stages, l-pairs packed on partition halves)
  softmax over history (batch-partitioned [128, 20] tiles)
  weighted pooling (DVE mul+reduce in gather layout)
  final DNN (K-chunked accumulating matmuls) -> y [1, 512]
"""

import numpy as np

import concourse.bass as bass
import concourse.bacc as bacc
import concourse.tile as tile
from concourse import mybir
from concourse.bass_utils import run_bass_kernel_spmd

f32 = mybir.dt.float32
f32r = mybir.dt.float32r
i32 = mybir.dt.int32
AF = mybir.ActivationFunctionType
ALU = mybir.AluOpType

# ---- problem sizes (hardcoded per the harness contract) ----
NCORES = 8
B = 4096
BC = B // NCORES          # 512 batch rows per core
NB = BC // 128            # 4 batch chunks of 128
NF = 20
V1 = 200001
D = 64
L = 19
L2 = 20                   # history padded to even count
G = L2 // 2               # 10 l-pairs
KC = 11                   # 1408 / 128 k-chunks for the final DNN
EC = NF * NB              # 80 emb gather chunks of 128 rows
HC = L * NB               # 76 hist gather chunks (l=19 pad is memset)
NEG_BIG = -1.0e30

# fp32r: same fp32 bits, 4x faster PE row rate at N>=256.
USE_FP32R = True

# How the indirect-DMA pairs multi-column offset tiles with output blocks:
# 'p' : index (p, c) -> out block (p, c)   [partition-major, matches CoreSim]
# 'c' : index order iterated column-major
GATHER_ORDER = "p"


def _mm(ap):
    return ap.bitcast(f32r) if USE_FP32R else ap


def _ap3(base_ap, dims):
    """Raw AP with explicit [step, count] dims on the same tensor/offset."""
    return bass.AP(base_ap.tensor, base_ap.offset, dims)


def build_program():
    nc = bacc.Bacc(trn_type="TRN2")

    dram = {}

    def din(name, shape, dt=f32):
        dram[name] = nc.dram_tensor(name, shape, dt, kind="ExternalInput")
        return dram[name]

    din("table", [NF * V1, D])
    din("eidx", [128, EC], i32)
    din("hidx", [128, HC], i32)
    din("denseT", [D, BC])
    din("lidx", [128, L2])
    din("lenf", [128, NB])
    din("ident", [128, 128])
    din("aw1q2", [128, D])
    din("aw1h2", [128, D])
    din("ab1_2", [128, 1])
    din("aw2bd", [128, 32])
    din("ab2_8", [128, 1])
    din("aw3p", [128, 8])
    din("dw1p", [128, KC, 128])
    din("db1", [128, 1])
    din("dw2", [128, D])
    din("db2", [D, 1])
    din("dw3", [D, 1])
    din("db3", [1, 1])
    y_dram = nc.dram_tensor("y", [1, BC], f32, kind="ExternalOutput")

    with tile.TileContext(nc) as tc:
        with (
            tc.tile_pool(name="persist", bufs=1) as P,
            tc.tile_pool(name="work", bufs=3) as W,
            tc.tile_pool(name="pbig", bufs=3, space="PSUM") as PB,
            tc.tile_pool(name="pt", bufs=3, space="PSUM") as PT,
            tc.tile_pool(name="ps", bufs=2, space="PSUM") as PS,
        ):
            # ---------------- input DMAs ----------------
            sb = {}

            def load(name, shape=None, dt=f32):
                t = P.tile(shape or list(dram[name].shape), dt, tag=name)
                nc.sync.dma_start(out=t[:], in_=dram[name][:])
                sb[name] = t
                return t

            load("eidx", dt=i32)
            load("hidx", dt=i32)
            t_ident = load("ident")
            load("aw1q2")
            load("aw1h2")
            load("ab1_2")
            load("aw2bd")
            load("ab2_8")
            load("aw3p")
            load("dw1p")
            load("db1")
            load("dw2")
            load("db2")
            load("dw3")
            load("db3")
            load("lidx")
            load("lenf")

            # XT: the final-DNN transposed input [128, KC, 512].
            # slot k=0: [emb f0 (query) ; dense], k=1..9: [f(2k-1) ; f(2k)],
            # k=10: [f19 ; pooled]
            t_XT = P.tile([128, KC, BC], f32, tag="XT")
            nc.sync.dma_start(out=t_XT[64:128, 0, :], in_=dram["denseT"][:])

            t_negbig = P.tile([128, 1], f32, tag="negbig")
            nc.vector.memset(t_negbig[:], NEG_BIG)

            # ---------------- gathers ----------------
            t_eraw = P.tile([128, EC, D], f32, tag="eraw")
            t_hraw = P.tile([128, NB * L2, D], f32, tag="hraw")
            nc.vector.memset(t_hraw[:, HC:, :], 0.0)

            def gather(out_ap, idx_ap):
                # HW indirect DMA supports exactly one offset per partition
                return nc.gpsimd.indirect_dma_start(
                    out=out_ap,
                    out_offset=None,
                    in_=dram["table"][:],
                    in_offset=bass.IndirectOffsetOnAxis(ap=idx_ap, axis=0),
                )

            # f0 (query) first so attention can start early, then hist
            # interleaved with the remaining emb features
            for c in range(4):
                gather(t_eraw[:, c, :], sb["eidx"][:, c : c + 1])
            for c in range(HC):
                gather(t_hraw[:, c, :], sb["hidx"][:, c : c + 1])
            for c in range(4, EC):
                gather(t_eraw[:, c, :], sb["eidx"][:, c : c + 1])

            # eraw viewed [p, bb, f, d]; hraw viewed [p, bb, l, d]
            eview = t_eraw[:].rearrange("p (f b) d -> p b f d", b=NB)
            hview = t_hraw[:].rearrange("p (l b) d -> p b l d", b=NB)

            cp_ctr = [0]

            def copy_alt(out_ap, in_ap):
                # split the psum->sbuf copy load between ACT and DVE
                if cp_ctr[0] % 2 == 0:
                    nc.scalar.copy(out=out_ap, in_=in_ap)
                else:
                    nc.vector.tensor_copy(out=out_ap, in_=in_ap)
                cp_ctr[0] += 1

            # ---------------- query transposes (f0 -> quT2 + XT0 top) ------
            t_quT2 = P.tile([128, BC], f32, tag="quT2")
            for bb in range(NB):
                src = eview[:, bb, 0, :]  # [128, 64]
                dup = _ap3(src, [src.ap[0], [0, 2], src.ap[1]])  # repeat twice
                pq = PT.tile([128, 128], f32, tag="pt")
                nc.tensor.matmul(
                    out=pq[:], lhsT=dup, rhs=t_ident[:], is_transpose=True
                )
                cols = slice(bb * 128, (bb + 1) * 128)
                copy_alt(t_quT2[:, cols], pq[:])
                copy_alt(t_XT[0:64, 0, cols], pq[0:64, :])

            # ---------------- hist transposes -> histT2 ----------------
            # histT2 [128, G, 512]: partitions 0:64 = d of even l, 64:128 odd l
            t_histT2 = P.tile([128, G, BC], f32, tag="histT2")
            for g in range(G):
                for bb in range(NB):
                    src = hview[:, bb, 2 * g : 2 * g + 2, :]  # [128, 2, 64]
                    ph = PT.tile([128, 128], f32, tag="pt")
                    nc.tensor.matmul(
                        out=ph[:], lhsT=src, rhs=t_ident[:], is_transpose=True
                    )
                    copy_alt(t_histT2[:, g, bb * 128 : (bb + 1) * 128], ph[:])

            # ---------------- emb transposes -> XT slots ----------------
            # pair (f=2k-1 -> top, f=2k -> bottom), same b-block
            emb_pairs = []
            for k in range(1, G):  # k=1..9 paired slots
                for bb in range(NB):
                    emb_pairs.append((k, bb))

            def emit_emb_pair(k, bb):
                f1 = 2 * k - 1
                src = eview[:, bb, f1 : f1 + 2, :]  # [128, 2, 64]
                pe_ = PT.tile([128, 128], f32, tag="pt")
                nc.tensor.matmul(
                    out=pe_[:], lhsT=src, rhs=t_ident[:], is_transpose=True
                )
                copy_alt(t_XT[:, k, bb * 128 : (bb + 1) * 128], pe_[:])

            # ---------------- attention mm1 + relu ----------------
            # h1T tiles [128, 512] per l-pair g: top=l even, bottom=l odd
            t_h1T = P.tile([128, G, BC], f32, tag="h1T")
            for g in range(G):
                p1 = PB.tile([128, BC], f32, tag="pb")
                for h in range(2):
                    rows = slice(h * 64, (h + 1) * 64)
                    nc.tensor.matmul(
                        out=p1[rows, :],
                        lhsT=_mm(sb["aw1q2"][rows, :]),
                        rhs=_mm(t_quT2[rows, :]),
                        start=True,
                        stop=False,
                    )
                    nc.tensor.matmul(
                        out=p1[rows, :],
                        lhsT=_mm(sb["aw1h2"][rows, :]),
                        rhs=_mm(t_histT2[rows, g, :]),
                        start=False,
                        stop=True,
                    )
                nc.scalar.activation(
                    out=t_h1T[:, g, :], in_=p1[:], func=AF.Relu,
                    bias=sb["ab1_2"][:],
                )
                # interleave emb transposes to keep them flowing
                for _ in range(4):
                    if emb_pairs:
                        emit_emb_pair(*emb_pairs.pop(0))

            while emb_pairs:
                emit_emb_pair(*emb_pairs.pop(0))

            # f19 transposes (top half of slot 10)
            for bb in range(NB):
                src = eview[:, bb, 19, :]  # [128, 64]
                pf = PT.tile([128, 128], f32, tag="pt")
                nc.tensor.matmul(
                    out=pf[0:64, :], lhsT=src, rhs=t_ident[:], is_transpose=True
                )
                copy_alt(t_XT[0:64, 10, bb * 128 : (bb + 1) * 128], pf[0:64, :])

            # ---------------- attention mm2 + relu ----------------
            # h2 psum tiles: quarter q=g%4 at partitions q*32..q*32+32
            NT = (G + 3) // 4  # 3 tiles (last half-filled)
            t_h2T = P.tile([128, NT, BC], f32, tag="h2T")
            p2_tiles = []
            for t in range(NT):
                p2 = PB.tile([128, BC], f32, tag="pb")
                p2_tiles.append(p2)
                for q in range(min(4, G - 4 * t)):
                    g = 4 * t + q
                    rows = slice(q * 32, (q + 1) * 32)
                    nc.tensor.matmul(
                        out=p2[rows, :],
                        lhsT=_mm(sb["aw2bd"][:, rows.start - q * 32 + 0 : 32] if False else sb["aw2bd"][:]),
                        rhs=_mm(t_h1T[:, g, :]),
                        start=True,
                        stop=True,
                    )
                nrows = 128 if G - 4 * t >= 4 else (G - 4 * t) * 32
                nc.scalar.activation(
                    out=t_h2T[0:nrows, t, :], in_=p2[0:nrows, :], func=AF.Relu,
                    bias=sb["ab2_8"][0:nrows, :],
                )

            # ---------------- attention mm3 (scores) ----------------
            t_scT = P.tile([8, NT, BC], f32, tag="scT")
            for t in range(NT):
                nl = min(8, L2 - 8 * t)  # 8, 8, 4
                p3 = PS.tile([8, BC], f32, tag="ps")
                nc.tensor.matmul(
                    out=p3[0:nl, :],
                    lhsT=_mm(sb["aw3p"][0 : (nl // 2) * 32, 0:nl]),
                    rhs=_mm(t_h2T[0 : (nl // 2) * 32, t, :]),
                    start=True,
                    stop=True,
                )
                nc.scalar.copy(out=t_scT[0:nl, t, :], in_=p3[0:nl, :])

            # ---------------- score transpose + softmax + pooling --------
            t_w = P.tile([128, L2 * NB], f32, tag="w")  # col = l*NB + bb
            w_view = t_w[:].rearrange("p (l b) -> p b l", b=NB)
            for bb in range(NB):
                cols = slice(bb * 128, (bb + 1) * 128)
                psc = PS.tile([128, L2], f32, tag="psc")
                for t in range(NT):
                    nl = min(8, L2 - 8 * t)
                    nc.tensor.matmul(
                        out=psc[:, 8 * t : 8 * t + nl],
                        lhsT=t_scT[0:nl, t, cols],
                        rhs=t_ident[0:nl, 0:nl],
                        is_transpose=True,
                    )
                # mask = lidx < len
                t_mask = W.tile([128, L2], f32, tag="mask")
                nc.vector.tensor_scalar(
                    out=t_mask[:], in0=sb["lidx"][:],
                    scalar1=sb["lenf"][:, bb : bb + 1], scalar2=None,
                    op0=ALU.is_lt,
                )
                t_sel = W.tile([128, L2], f32, tag="sel")
                nc.vector.select(
                    out=t_sel[:], mask=t_mask[:], on_true=psc[:],
                    on_false=t_negbig[:].to_broadcast([128, L2]),
                )
                t_nmax = W.tile([128, 1], f32, tag="nmax")
                nc.vector.tensor_reduce(
                    out=t_nmax[:], in_=t_sel[:], axis=mybir.AxisListType.X,
                    op=ALU.max, negate=True,
                )
                t_p = W.tile([128, L2], f32, tag="p")
                t_rs = W.tile([128, 1], f32, tag="rs")
                nc.scalar.activation(
                    out=t_p[:], in_=t_sel[:], func=AF.Exp,
                    bias=t_nmax[:], accum_out=t_rs[:],
                )
                t_winv = W.tile([128, 1], f32, tag="winv")
                nc.vector.reciprocal(out=t_winv[:], in_=t_rs[:])
                nc.vector.tensor_scalar(
                    out=w_view[:, bb, :], in0=t_p[:], scalar1=t_winv[:],
                    scalar2=None, op0=ALU.mult,
                )

                # pooling for this b-chunk: tmp = hist * w, reduce over l
                t_tmp = W.tile([128, L2, D], f32, tag="ptmp")
                hist_bb = hview[:, bb, :, :]  # [128, L2, 64]
                w_bb = w_view[:, bb, :]       # [128, 20] (step NB)
                w_bc = _ap3(w_bb, [w_bb.ap[0], w_bb.ap[1], [0, D]])
                nc.vector.tensor_tensor(
                    out=t_tmp[:], in0=hist_bb, in1=w_bc, op=ALU.mult
                )
                t_pool = W.tile([128, D], f32, tag="pool")
                nc.vector.tensor_reduce(
                    out=t_pool[:],
                    in_=t_tmp[:].rearrange("p l d -> p d l"),
                    axis=mybir.AxisListType.X,
                    op=ALU.add,
                )
                # transpose pooled -> XT slot 10 bottom
                pp = PT.tile([128, 128], f32, tag="pt")
                nc.tensor.matmul(
                    out=pp[64:128, :], lhsT=t_pool[:], rhs=t_ident[:],
                    is_transpose=True,
                )
                nc.scalar.copy(out=t_XT[64:128, 10, cols], in_=pp[64:128, :])

            # ---------------- final DNN ----------------
            pd1 = PB.tile([128, BC], f32, tag="pb")
            for k in range(KC):
                nc.tensor.matmul(
                    out=pd1[:],
                    lhsT=_mm(sb["dw1p"][:, k, :]),
                    rhs=_mm(t_XT[:, k, :]),
                    start=(k == 0),
                    stop=(k == KC - 1),
                )
            t_x2 = P.tile([128, BC], f32, tag="x2")
            nc.scalar.activation(
                out=t_x2[:], in_=pd1[:], func=AF.Relu, bias=sb["db1"][:]
            )
            pd2 = PB.tile([128, BC], f32, tag="pb")
            nc.tensor.matmul(
                out=pd2[0:64, :], lhsT=_mm(sb["dw2"][:]), rhs=_mm(t_x2[:]),
                start=True, stop=True,
            )
            t_x3 = P.tile([64, BC], f32, tag="x3")
            nc.scalar.activation(
                out=t_x3[:], in_=pd2[0:64, :], func=AF.Relu, bias=sb["db2"][:]
            )
            pd3 = PS.tile([1, BC], f32, tag="ps")
            nc.tensor.matmul(
                out=pd3[:], lhsT=_mm(sb["dw3"][:]), rhs=_mm(t_x3[:]),
                start=True, stop=True,
            )
            t_y = P.tile([1, BC], f32, tag="y")
            nc.vector.tensor_scalar(
                out=t_y[:], in0=pd3[:], scalar1=sb["db3"][0:1, :],
                scalar2=None, op0=ALU.add,
            )
            nc.sync.dma_start(out=y_dram[:], in_=t_y[:])

    nc.compile()
    return nc


# ---------------------------------------------------------------------------
# host-side prep
# ---------------------------------------------------------------------------

def _colmajor128(a):
    """[N] -> [128, N//128] with element r at (r%128, r//128)."""
    n = a.shape[0]
    return np.ascontiguousarray(a.reshape(n // 128, 128).T)


def make_core_inputs(inputs, c):
    """Build the per-core input map for core c from full-problem inputs."""
    bsl = slice(c * BC, (c + 1) * BC)
    sparse = np.asarray(inputs["sparse_inputs"][bsl], dtype=np.int64)
    hist = np.asarray(inputs["history"][bsl], dtype=np.int64)
    hlen = np.asarray(inputs["history_length"][bsl], dtype=np.int64)
    dense = np.asarray(inputs["dense_inputs"][bsl], dtype=np.float32)

    # emb gather rows r = f*BC + b ; hist rows r = l*BC + b
    eflat = (sparse.T + (np.arange(NF, dtype=np.int64) * V1)[:, None]).ravel()
    hflat = (hist.T + (np.arange(1, L + 1, dtype=np.int64) * V1)[:, None]).ravel()
    eidx = _colmajor128(eflat).astype(np.int32)
    hidx = _colmajor128(hflat).astype(np.int32)
    if GATHER_ORDER == "c":
        # device pairs indices column-major: pre-permute so data lands as 'p'
        ec = eidx.shape[1]
        eidx = eflat.reshape(ec, 128).T.T.reshape(ec, 128).T  # placeholder
        raise NotImplementedError

    aw1 = np.asarray(inputs["aw1"], dtype=np.float32)
    aw2 = np.asarray(inputs["aw2"], dtype=np.float32)
    aw3 = np.asarray(inputs["aw3"], dtype=np.float32)
    ab1 = np.asarray(inputs["ab1"], dtype=np.float32)
    ab2 = np.asarray(inputs["ab2"], dtype=np.float32)
    dw1 = np.asarray(inputs["dw1"], dtype=np.float32)
    dw2 = np.asarray(inputs["dw2"], dtype=np.float32)
    dw3 = np.asarray(inputs["dw3"], dtype=np.float32)

    aw1q2 = np.concatenate([aw1[:D], aw1[:D]], axis=0)
    aw1h2 = np.concatenate([aw1[D:], aw1[D:]], axis=0)
    ab1_2 = np.concatenate([ab1, ab1])[:, None]
    aw2bd = np.zeros((128, 32), np.float32)
    aw2bd[0:64, 0:16] = aw2
    aw2bd[64:128, 16:32] = aw2
    ab2_8 = np.tile(ab2, 8)[:, None]
    aw3p = np.zeros((128, 8), np.float32)
    for q in range(4):
        for h in range(2):
            aw3p[q * 32 + h * 16 : q * 32 + h * 16 + 16, q * 2 + h] = aw3[:, 0]

    # dw1 rows permuted: slot0 = [emb f0 ; dense]
    dw1_perm = dw1.copy()
    dw1_perm[0:64] = dw1[64:128]
    dw1_perm[64:128] = dw1[0:64]
    dw1p = np.ascontiguousarray(
        dw1_perm.reshape(KC, 128, 128).transpose(1, 0, 2)
    )

    lidx = np.broadcast_to(
        np.arange(L2, dtype=np.float32)[None, :], (128, L2)
    ).copy()
    lenf = _colmajor128(hlen.astype(np.float32))

    return {
        "table": inputs["_table_flat"],
        "eidx": eidx,
        "hidx": hidx,
        "denseT": np.ascontiguousarray(dense.T),
        "lidx": lidx,
        "lenf": np.ascontiguousarray(lenf),
        "ident": np.eye(128, dtype=np.float32),
        "aw1q2": np.ascontiguousarray(aw1q2),
        "aw1h2": np.ascontiguousarray(aw1h2),
        "ab1_2": np.ascontiguousarray(ab1_2),
        "aw2bd": aw2bd,
        "ab2_8": np.ascontiguousarray(ab2_8),
        "aw3p": aw3p,
        "dw1p": dw1p,
        "db1": np.asarray(inputs["db1"], np.float32)[:, None],
        "dw2": dw2,
        "db2": np.asarray(inputs["db2"], np.float32)[:, None],
        "dw3": dw3,
        "db3": np.asarray(inputs["db3"], np.float32).reshape(1, 1),
        "lidx_unused": None,
    }


def prep_all_core_inputs(inputs):
    inputs = dict(inputs)
    inputs["_table_flat"] = np.ascontiguousarray(
        np.asarray(inputs["emb_tables"], dtype=np.float32).reshape(NF * V1, D)
    )
    maps = []
    for c in range(NCORES):
        m = make_core_inputs(inputs, c)
        m.pop("lidx_unused", None)
        maps.append(m)
    return maps


_CACHED_NC = None


def kernel(**inputs) -> np.ndarray:
    global _CACHED_NC
    if _CACHED_NC is None:
        _CACHED_NC = build_program()
    maps = prep_all_core_inputs(inputs)
    res = run_bass_kernel_spmd(_CACHED_NC, maps, core_ids=list(range(NCORES)))
    return np.concatenate([r["y"][0] for r in res.results]).astype(np.float32)
